# revision 5
# baseline (speedup 1.0000x reference)
"""PhysNet GNN message passing on 8 trn2 NeuronCores (Bass/Tile SPMD).

Strategy: shard 50000 atoms across 8 cores (6250 each). Pairs are grouped by
destination atom into 128-atom windows (idx_i sorted), padded to a uniform
per-window chunk budget so all cores run one SPMD program. Per block:
dense layers in transposed layout (x_T [F, atoms]); xj rows AllGathered into
a shared 50000x128 table; per 128-pair chunk: indirect-DMA gather of xj rows,
g = descr @ Wg, msg = g*xj, scatter-add via one-hot matmul into window PSUM.
ssp(x)=softplus(x)-log2 is approximated as (silu(kx) - k*log2*tanh^2(cx))/k
(max abs err 8.6e-4; no Softplus/Ln table exists on trn2); the 1/k is folded
into the next layer's weights on the host, so device activations carry a k*
scale.
"""
import os
import sys
import time as _time
sys.path.insert(0, "/opt/trn_rl_repo")
import numpy as np
import concourse.bass as bass
import concourse.bacc as bacc
import concourse.mybir as mybir
import concourse.tile as tile
from concourse import bass_utils
from concourse.masks import make_identity

NC = 8
N_ATOMS = 50000
N_PAIRS = 1000000
NA = N_ATOMS // NC          # 6250 atoms per core
F = 128
K = 64
B = 5
NRI, NRF = 3, 2
P = 128
NW = (NA + P - 1) // P      # 49 windows of 128 atoms
LOG2 = float(np.log(2.0))

# fitted ssp approximation params
KA = 1.04378291
CA = 0.43927521
SQ = float(np.sqrt(KA * LOG2))   # Square scale so C = k*log2*tanh^2

_f32 = mybir.dt.float32
_i32 = mybir.dt.int32

AF = mybir.ActivationFunctionType
OP = mybir.AluOpType

COL_T = 512   # dense col tile


def _ssp_scaled(nc, sp, out_sbuf, z, bias_k=None, bias_c=None, scale=1.0):
    """out = k*ssp(scale*z + b) given ACT biases k*(scale b) pre-mult.
    z may be PSUM or SBUF. bias_k/bias_c are [128,1] APs already scaled by
    k and c respectively (or None). Emits Silu+Tanh+Square(ACT) + sub(DVE)."""
    shp = [z.shape[0], z.shape[1]]
    a_t = sp.tile(shp, _f32, tag="ssp_a")
    b_t = sp.tile(shp, _f32, tag="ssp_b")
    c_t = sp.tile(shp, _f32, tag="ssp_c")
    kw_a = dict(scale=KA * scale) if bias_k is None else dict(scale=KA * scale, bias=bias_k)
    kw_b = dict(scale=CA * scale) if bias_c is None else dict(scale=CA * scale, bias=bias_c)
    nc.scalar.activation(a_t[:], z, AF.Silu, **kw_a)
    nc.scalar.activation(b_t[:], z, AF.Tanh, **kw_b)
    nc.scalar.activation(c_t[:], b_t[:], AF.Square, scale=SQ)
    nc.vector.tensor_tensor(out=out_sbuf, in0=a_t[:], in1=c_t[:], op=OP.subtract)


def build(w_ch):
    """Build the SPMD program. w_ch = uniform chunks per window."""
    nc = bacc.Bacc("TRN2", target_bir_lowering=False, debug=False, num_devices=NC)
    TCH = NW * w_ch                      # chunks per block per core
    x0 = nc.dram_tensor("x0", [P, NA], _f32, kind="ExternalInput")
    descr = nc.dram_tensor("descr", [K, TCH * P], _f32, kind="ExternalInput")
    idxs = nc.dram_tensor("idxs", [P, TCH], _i32, kind="ExternalInput")
    offs = nc.dram_tensor("offs", [P, TCH], _f32, kind="ExternalInput")
    iota = nc.dram_tensor("iota", [P, P], _f32, kind="ExternalInput")
    wall = nc.dram_tensor("wall", [B * 13 * P, P], _f32, kind="ExternalInput")
    wg_all = nc.dram_tensor("wg_all", [B * K, P], _f32, kind="ExternalInput")
    biasT = nc.dram_tensor("biasT", [P, B * 32], _f32, kind="ExternalInput")
    xout = nc.dram_tensor("xout", [B, P, NA], _f32, kind="ExternalOutput")
    xj_full = nc.dram_tensor("xj_full", [N_ATOMS, P], _f32,
                             kind="Internal", addr_space="Shared")

    # weight row-block index within wall (per block): Wi,Wj,Wr1x3,Wr2x3,Wout,Wf1x2,Wf2x2
    def wslice(b, j):
        r = (b * 13 + j) * P
        return wall[r:r + P, :]

    # bias column index within biasT (per block, 32 slots):
    # 0:k*bi 1:c*bi 2:k*bj 3:c*bj 4..9: (k,c)*br1 r=0..2  10..12: k*(br2+fold) r
    # 13: bout 14: u 15..18: (k,c)*bf1 r=0..1  19..20: bf2 r  21: unused
    def bcol(b, j):
        return b * 32 + j

    with tile.TileContext(nc) as tc:
        with tc.tile_pool(name="pers", bufs=1) as pp, \
             tc.tile_pool(name="sp", bufs=2) as sp, \
             tc.tile_pool(name="dp", bufs=2) as dp, \
             tc.tile_pool(name="wp", bufs=2) as wp, \
             tc.tile_pool(name="gp", bufs=3) as gpool, \
             tc.tile_pool(name="ps", bufs=2, space="PSUM") as ps, \
             tc.tile_pool(name="dr", bufs=1, space="DRAM") as dr:
            x_t = pp.tile([P, NA], _f32, tag="x")
            xi_t = pp.tile([P, NA], _f32, tag="xi")
            xjt_t = pp.tile([P, NA], _f32, tag="xjt")
            m_t = pp.tile([P, NA], _f32, tag="m")
            xa_t = m_t  # xa dead before m is written; share the slot
            idx_sb = pp.tile([P, TCH], _i32, tag="idx")
            off_sb = pp.tile([P, TCH], _f32, tag="off")
            iota_sb = pp.tile([P, P], _f32, tag="iota")
            bias_sb = pp.tile([P, B * 32], _f32, tag="bias")
            ident = pp.tile([P, P], _f32, tag="ident")
            nc.sync.dma_start(out=x_t[:], in_=x0[:])
            nc.sync.dma_start(out=idx_sb[:], in_=idxs[:])
            nc.sync.dma_start(out=off_sb[:], in_=offs[:])
            nc.sync.dma_start(out=iota_sb[:], in_=iota[:])
            nc.sync.dma_start(out=bias_sb[:], in_=biasT[:])
            make_identity(nc, ident[:])

            ntile = (NA + COL_T - 1) // COL_T
            tiles = [(t * COL_T, min(COL_T, NA - t * COL_T)) for t in range(ntile)]

            def dense(dst, src, widx, bk, bc, b):
                """dst = k*ssp(src @ W + bias) tile-by-tile (transposed layout)."""
                w_sb = wp.tile([P, P], _f32, tag="w")
                nc.sync.dma_start(out=w_sb[:], in_=wslice(b, widx))
                for (c0, cn) in tiles:
                    z = ps.tile([P, COL_T], _f32, tag="z")
                    nc.tensor.matmul(z[:, :cn], lhsT=w_sb[:], rhs=src[:, c0:c0 + cn],
                                     start=True, stop=True)
                    _ssp_scaled(nc, sp, dst[:, c0:c0 + cn], z[:, :cn],
                                bias_k=bias_sb[:, bcol(b, bk):bcol(b, bk) + 1],
                                bias_c=bias_sb[:, bcol(b, bc):bcol(b, bc) + 1])

            for b in range(B):
                # xa = k*ssp(x)
                for (c0, cn) in tiles:
                    _ssp_scaled(nc, sp, xa_t[:, c0:c0 + cn], x_t[:, c0:c0 + cn])
                dense(xi_t, xa_t, 0, 0, 1, b)   # xi (scaled)
                dense(xjt_t, xa_t, 1, 2, 3, b)  # xj (scaled, transposed)
                # transpose xj to rows and write local shard of the table
                xj_loc = dr.tile([NA, P], _f32, tag="xjloc")
                for w in range(NW):
                    wa = min(P, NA - w * P)
                    tp = ps.tile([P, P], _f32, tag="tp")
                    nc.tensor.transpose(out=tp[:wa, :], in_=xjt_t[:, w * P:w * P + wa],
                                        identity=ident[:])
                    rows = sp.tile([P, P], _f32, tag="rows")
                    nc.vector.tensor_copy(out=rows[:wa, :], in_=tp[:wa, :])
                    nc.sync.dma_start(out=xj_loc[w * P:w * P + wa, :], in_=rows[:wa, :])
                nc.gpsimd.collective_compute(
                    "AllGather", OP.bypass,
                    replica_groups=[list(range(NC))],
                    ins=[xj_loc[:]], outs=[xj_full[:]])

                # pair stream
                wg_sb = wp.tile([K, P], _f32, tag="wg")
                nc.sync.dma_start(out=wg_sb[:], in_=wg_all[b * K:(b + 1) * K, :])
                for w in range(NW):
                    wa = min(P, NA - w * P)
                    dt = dp.tile([K, w_ch * P], _f32, tag="descr")
                    nc.sync.dma_start(
                        out=dt[:], in_=descr[:, w * w_ch * P:(w + 1) * w_ch * P])
                    macc = ps.tile([P, P], _f32, tag="macc")
                    for c in range(w_ch):
                        ci = w * w_ch + c
                        gt = gpool.tile([P, P], _f32, tag="gt")
                        nc.gpsimd.indirect_dma_start(
                            out=gt[:], out_offset=None, in_=xj_full[:],
                            in_offset=bass.IndirectOffsetOnAxis(
                                ap=idx_sb[:, ci:ci + 1], axis=0))
                        gps = ps.tile([P, P], _f32, tag="gps")
                        nc.tensor.matmul(gps[:], lhsT=dt[:, c * P:(c + 1) * P],
                                         rhs=wg_sb[:], start=True, stop=True)
                        s_t = sp.tile([P, P], _f32, tag="s")
                        nc.vector.tensor_scalar(
                            out=s_t[:], in0=iota_sb[:],
                            scalar1=off_sb[:, ci:ci + 1], scalar2=None,
                            op0=OP.is_equal)
                        msg = sp.tile([P, P], _f32, tag="msg")
                        nc.vector.tensor_tensor(out=msg[:], in0=gps[:], in1=gt[:],
                                                op=OP.mult)
                        nc.tensor.matmul(macc[:], lhsT=msg[:], rhs=s_t[:],
                                         start=(c == 0), stop=(c == w_ch - 1))
                    nc.vector.tensor_tensor(
                        out=m_t[:, w * P:w * P + wa], in0=macc[:, :wa],
                        in1=xi_t[:, w * P:w * P + wa], op=OP.add)

                # residual (interaction) x3 — all on m (scaled)
                for r in range(NRI):
                    w1 = wp.tile([P, P], _f32, tag="w")
                    nc.sync.dma_start(out=w1[:], in_=wslice(b, 2 + r))
                    w2 = wp.tile([P, P], _f32, tag="w")
                    nc.sync.dma_start(out=w2[:], in_=wslice(b, 5 + r))
                    for (c0, cn) in tiles:
                        t1 = sp.tile([P, COL_T], _f32, tag="t1")
                        _ssp_scaled(nc, sp, t1[:, :cn], m_t[:, c0:c0 + cn],
                                    scale=1.0 / KA)  # m is k-scaled
                        z1 = ps.tile([P, COL_T], _f32, tag="z")
                        nc.tensor.matmul(z1[:, :cn], lhsT=w1[:], rhs=t1[:, :cn],
                                         start=True, stop=True)
                        t2 = sp.tile([P, COL_T], _f32, tag="t2")
                        _ssp_scaled(nc, sp, t2[:, :cn], z1[:, :cn],
                                    bias_k=bias_sb[:, bcol(b, 4 + 2 * r):bcol(b, 4 + 2 * r) + 1],
                                    bias_c=bias_sb[:, bcol(b, 5 + 2 * r):bcol(b, 5 + 2 * r) + 1])
                        z2 = ps.tile([P, COL_T], _f32, tag="z")
                        nc.tensor.matmul(z2[:, :cn], lhsT=w2[:], rhs=t2[:, :cn],
                                         start=True, stop=True)
                        t3 = sp.tile([P, COL_T], _f32, tag="t3")
                        nc.vector.tensor_scalar(
                            out=t3[:, :cn], in0=z2[:, :cn], scalar1=KA,
                            scalar2=bias_sb[:, bcol(b, 10 + r):bcol(b, 10 + r) + 1],
                            op0=OP.mult, op1=OP.add)
                        nc.vector.tensor_tensor(out=m_t[:, c0:c0 + cn],
                                                in0=m_t[:, c0:c0 + cn],
                                                in1=t3[:, :cn], op=OP.add)
                # x = u*x + ssp(m)@Wout + bout   (ssp(m) scaled; Wout pre-divided)
                wo = wp.tile([P, P], _f32, tag="w")
                nc.sync.dma_start(out=wo[:], in_=wslice(b, 8))
                for (c0, cn) in tiles:
                    mp = sp.tile([P, COL_T], _f32, tag="t1")
                    _ssp_scaled(nc, sp, mp[:, :cn], m_t[:, c0:c0 + cn], scale=1.0 / KA)
                    z = ps.tile([P, COL_T], _f32, tag="z")
                    nc.tensor.matmul(z[:, :cn], lhsT=wo[:], rhs=mp[:, :cn],
                                     start=True, stop=True)
                    ux = sp.tile([P, COL_T], _f32, tag="t2")
                    nc.vector.tensor_scalar(
                        out=ux[:, :cn], in0=x_t[:, c0:c0 + cn],
                        scalar1=bias_sb[:, bcol(b, 14):bcol(b, 14) + 1],
                        scalar2=bias_sb[:, bcol(b, 13):bcol(b, 13) + 1],
                        op0=OP.mult, op1=OP.add)
                    nc.vector.tensor_tensor(out=x_t[:, c0:c0 + cn], in0=ux[:, :cn],
                                            in1=z[:, :cn], op=OP.add)
                # residual (feature) x2 — on x (true scale)
                for r in range(NRF):
                    w1 = wp.tile([P, P], _f32, tag="w")
                    nc.sync.dma_start(out=w1[:], in_=wslice(b, 9 + r))
                    w2 = wp.tile([P, P], _f32, tag="w")
                    nc.sync.dma_start(out=w2[:], in_=wslice(b, 11 + r))  # careful map
                    for (c0, cn) in tiles:
                        t1 = sp.tile([P, COL_T], _f32, tag="t1")
                        _ssp_scaled(nc, sp, t1[:, :cn], x_t[:, c0:c0 + cn])
                        z1 = ps.tile([P, COL_T], _f32, tag="z")
                        nc.tensor.matmul(z1[:, :cn], lhsT=w1[:], rhs=t1[:, :cn],
                                         start=True, stop=True)
                        t2 = sp.tile([P, COL_T], _f32, tag="t2")
                        _ssp_scaled(nc, sp, t2[:, :cn], z1[:, :cn],
                                    bias_k=bias_sb[:, bcol(b, 15 + 2 * r):bcol(b, 15 + 2 * r) + 1],
                                    bias_c=bias_sb[:, bcol(b, 16 + 2 * r):bcol(b, 16 + 2 * r) + 1])
                        z2 = ps.tile([P, COL_T], _f32, tag="z")
                        nc.tensor.matmul(z2[:, :cn], lhsT=w2[:], rhs=t2[:, :cn],
                                         start=True, stop=True)
                        t3 = sp.tile([P, COL_T], _f32, tag="t3")
                        nc.vector.tensor_scalar(
                            out=t3[:, :cn], in0=z2[:, :cn], scalar1=1.0,
                            scalar2=bias_sb[:, bcol(b, 19 + r):bcol(b, 19 + r) + 1],
                            op0=OP.mult, op1=OP.add)
                        nc.vector.tensor_tensor(out=x_t[:, c0:c0 + cn],
                                                in0=x_t[:, c0:c0 + cn],
                                                in1=t3[:, :cn], op=OP.add)
                nc.sync.dma_start(out=xout[b, :, :], in_=x_t[:])
    nc.compile()
    return nc


def _run_spmd_timed(nc, in_maps, n_iters):
    """Mirror bass2jax.run_bass_via_pjrt's multi-core path, but keep inputs
    device-resident and time repeat executions (outputs are donated, so each
    timing iteration gets fresh device-placed zero buffers outside the timed
    region). Returns (per_core_results, best_exec_ns)."""
    import jax
    from jax.experimental.shard_map import shard_map
    from jax.sharding import Mesh, PartitionSpec, NamedSharding
    from concourse import bass2jax as b2j

    b2j.install_neuronx_cc_hook()
    partition_name = (nc.partition_id_tensor.name
                      if nc.partition_id_tensor else None)
    in_names, out_names, out_avals, zero_shapes = [], [], [], []
    for alloc in nc.m.functions[0].allocations:
        if not isinstance(alloc, mybir.MemoryLocationSet):
            continue
        name = alloc.memorylocations[0].name
        if alloc.kind == "ExternalInput":
            if name != partition_name:
                in_names.append(name)
        elif alloc.kind == "ExternalOutput":
            shape = tuple(alloc.tensor_shape)
            dtype = mybir.dt.np(alloc.dtype)
            out_names.append(name)
            out_avals.append(jax.core.ShapedArray(shape, dtype))
            zero_shapes.append((shape, dtype))
    n_params = len(in_names)
    n_outs = len(out_avals)
    in_names.extend(out_names)
    if partition_name is not None:
        in_names.append(partition_name)

    def _body(*args):
        operands = list(args)
        if partition_name is not None:
            operands.append(b2j.partition_id_tensor())
        outs = b2j._bass_exec_p.bind(
            *operands, out_avals=tuple(out_avals), in_names=tuple(in_names),
            out_names=tuple(out_names), lowering_input_output_aliases=(),
            sim_require_finite=True, sim_require_nnan=True, nc=nc)
        return tuple(outs)

    donate = tuple(range(n_params, n_params + n_outs))
    devices = jax.devices()[:NC]
    mesh = Mesh(np.asarray(devices), ("core",))
    specs = (PartitionSpec("core"),)
    sharded = jax.jit(
        shard_map(_body, mesh=mesh, in_specs=specs * (n_params + n_outs),
                  out_specs=specs * n_outs, check_rep=False),
        donate_argnums=donate, keep_unused=True)
    sh = NamedSharding(mesh, PartitionSpec("core"))
    names = in_names[:n_params]
    dev_in = [jax.device_put(
        np.concatenate([np.asarray(m[nm]) for m in in_maps], axis=0), sh)
        for nm in names]
    for a in dev_in:
        a.block_until_ready()

    def fresh_zeros():
        zs = [jax.device_put(np.zeros((NC * s[0], *s[1:]), d), sh)
              for s, d in zero_shapes]
        for z in zs:
            z.block_until_ready()
        return zs

    outs = sharded(*dev_in, *fresh_zeros())
    results_np = [np.asarray(o) for o in outs]
    times = []
    for _ in range(n_iters):
        zs = fresh_zeros()
        t0 = _time.perf_counter()
        o2 = sharded(*dev_in, *zs)
        for o in o2:
            o.block_until_ready()
        times.append(_time.perf_counter() - t0)
        del o2
    exec_ns = int(min(times) * 1e9) if times else -1
    if times:
        print("timed iters (ms):", [round(t * 1e3, 3) for t in times])
    per_core = [
        {nm: results_np[i].reshape(NC, *out_avals[i].shape)[c]
         for i, nm in enumerate(out_names)}
        for c in range(NC)
    ]
    return per_core, exec_ns


def kernel(**inputs):
    feats = np.asarray(inputs["features"], np.float32)
    cutoffs = np.asarray(inputs["cutoffs"], np.float32)
    rbfs = np.asarray(inputs["rbfs"], np.float32)
    idx_i = np.asarray(inputs["idx_i"]).astype(np.int64)
    idx_j = np.asarray(inputs["idx_j"]).astype(np.int64)
    W = {k: np.asarray(inputs[k], np.float32) for k in
         ["Wg", "Wi", "bi", "Wj", "bj", "Wr1", "br1", "Wr2", "br2",
          "Wout", "bout", "u", "Wf1", "bf1", "Wf2", "bf2"]}
    CC = -8.9582e-4  # ssp approx constant; folded into consumer biases below

    descr_full = cutoffs[:, None] * rbfs                      # [Pairs, K]

    # ---- shard pairs by destination atom core & window; compute budget ----
    bounds = np.searchsorted(idx_i, np.arange(0, N_ATOMS + 1, NA))
    win_of = (idx_i % NA) // P                               # window within core
    cnts = np.zeros((NC, NW), np.int64)
    for c in range(NC):
        s, e = bounds[c], bounds[c + 1]
        cnts[c] = np.bincount(win_of[s:e], minlength=NW)
    w_ch = int(np.ceil(cnts.max() / P))
    TCH = NW * w_ch

    in_maps = []
    for c in range(NC):
        s, e = bounds[c], bounds[c + 1]
        d = np.zeros((TCH * P, K), np.float32)
        ji = np.zeros((TCH * P,), np.int32)
        of = np.zeros((TCH * P,), np.float32)
        pos = s
        for w in range(NW):
            n = cnts[c, w]
            base = w * w_ch * P
            d[base:base + n] = descr_full[pos:pos + n]
            ji[base:base + n] = idx_j[pos:pos + n]
            of[base:base + n] = (idx_i[pos:pos + n] % NA) - w * P
            pos += n
        # device layouts
        descr_t = np.ascontiguousarray(d.T)                   # [K, TCH*P]
        idx_t = np.ascontiguousarray(ji.reshape(TCH, P).T)    # [P, TCH]
        off_t = np.ascontiguousarray(of.reshape(TCH, P).T)    # [P, TCH]
        x0 = np.ascontiguousarray(feats[c * NA:(c + 1) * NA].T)
        in_maps.append(dict(x0=x0, descr=descr_t, idxs=idx_t, offs=off_t))

    iota = np.broadcast_to(np.arange(P, dtype=np.float32), (P, P)).copy()
    # ---- weights: fold 1/KA into consumers of scaled activations ----
    inv = 1.0 / KA
    wall = np.zeros((B, 13, P, P), np.float32)
    wg_all = np.zeros((B, K, P), np.float32)
    biasT = np.zeros((B, 32, P), np.float32)
    for b in range(B):
        wall[b, 0] = W["Wi"][b] * inv
        wall[b, 1] = W["Wj"][b] * inv
        for r in range(NRI):
            wall[b, 2 + r] = W["Wr1"][b, r] * inv
            wall[b, 5 + r] = W["Wr2"][b, r] * inv
        wall[b, 8] = W["Wout"][b] * inv
        for r in range(NRF):
            wall[b, 9 + r] = W["Wf1"][b, r] * inv
            wall[b, 11 + r] = W["Wf2"][b, r] * inv
        wg_all[b] = W["Wg"][b]
        bi_e = W["bi"][b] + CC * W["Wi"][b].sum(0)
        bj_e = W["bj"][b] + CC * W["Wj"][b].sum(0)
        biasT[b, 0] = KA * bi_e
        biasT[b, 1] = CA * bi_e
        biasT[b, 2] = KA * bj_e
        biasT[b, 3] = CA * bj_e
        for r in range(NRI):
            br1_e = W["br1"][b, r] + CC * W["Wr1"][b, r].sum(0)
            br2_e = W["br2"][b, r] + CC * W["Wr2"][b, r].sum(0)
            biasT[b, 4 + 2 * r] = KA * br1_e
            biasT[b, 5 + 2 * r] = CA * br1_e
            biasT[b, 10 + r] = KA * br2_e
        biasT[b, 13] = W["bout"][b] + CC * W["Wout"][b].sum(0)
        biasT[b, 14] = W["u"][b]
        for r in range(NRF):
            bf1_e = W["bf1"][b, r] + CC * W["Wf1"][b, r].sum(0)
            biasT[b, 15 + 2 * r] = KA * bf1_e
            biasT[b, 16 + 2 * r] = CA * bf1_e
            biasT[b, 19 + r] = W["bf2"][b, r] + CC * W["Wf2"][b, r].sum(0)
    shared = dict(iota=iota,
                  wall=wall.reshape(B * 13 * P, P),
                  wg_all=wg_all.reshape(B * K, P),
                  biasT=np.ascontiguousarray(
                      biasT.reshape(B * 32, P).T))
    for m in in_maps:
        m.update(shared)

    nc = build(w_ch)
    global LAST_EXEC_NS
    n_iters = int(os.environ.get("PHYS_TIME_ITERS", "0"))
    if n_iters > 0:
        results, LAST_EXEC_NS = _run_spmd_timed(nc, in_maps, n_iters)
    else:
        res = bass_utils.run_bass_kernel_spmd(
            nc, in_maps, core_ids=list(range(NC)))
        results = res.results
        LAST_EXEC_NS = res.exec_time_ns if res.exec_time_ns is not None else -1
    out = np.empty((B, N_ATOMS, F), np.float32)
    for c in range(NC):
        slab = results[c]["xout"]              # [B, 128, NA]
        out[:, c * NA:(c + 1) * NA, :] = np.transpose(slab, (0, 2, 1))
    return out



# revision 19
# speedup vs baseline: 1.8423x; 1.8423x over previous
"""PhysNet GNN message passing on 8 trn2 NeuronCores (Bass/Tile SPMD).

Strategy: shard 50000 atoms across 8 cores (6250 each). Pairs are grouped by
destination atom into 128-atom windows (idx_i sorted), padded to a uniform
per-window chunk budget so all cores run one SPMD program. Per block:
dense layers in transposed layout (x_T [F, atoms]); xj rows AllGathered into
a shared 50000x128 table; per 128-pair chunk: indirect-DMA gather of xj rows,
g = descr @ Wg, msg = g*xj, scatter-add via one-hot matmul into window PSUM.
ssp(x)=softplus(x)-log2 is approximated as (silu(kx) - k*log2*tanh^2(cx))/k
(max abs err 8.6e-4; no Softplus/Ln table exists on trn2); the 1/k is folded
into the next layer's weights on the host, so device activations carry a k*
scale.
"""
import os
import sys
import time as _time
sys.path.insert(0, "/opt/trn_rl_repo")
import numpy as np
import concourse.bass as bass
import concourse.bacc as bacc
import concourse.mybir as mybir
import concourse.tile as tile
from concourse import bass_utils
from concourse.masks import make_identity

NC = 8
N_ATOMS = 50000
N_PAIRS = 1000000
NA = N_ATOMS // NC          # 6250 atoms per core
F = 128
K = 64
B = 5
NRI, NRF = 3, 2
P = 128
NW = (NA + P - 1) // P      # 49 windows of 128 atoms
LOG2 = float(np.log(2.0))

# fitted ssp approximation params
KA = 1.04378291
CA = 0.43927521
SQ = float(np.sqrt(KA * LOG2))   # Square scale so C = k*log2*tanh^2

_f32 = mybir.dt.float32
_i32 = mybir.dt.int32

AF = mybir.ActivationFunctionType
OP = mybir.AluOpType

COL_T = 512   # dense col tile


def _ssp_scaled(nc, sp, out_sbuf, z, bias_k=None, bias_c=None, scale=1.0):
    """out = k*ssp(scale*z + b) given ACT biases k*(scale b) pre-mult.
    z may be PSUM or SBUF. bias_k/bias_c are [128,1] APs already scaled by
    k and c respectively (or None). Emits Silu+Tanh+Square(ACT) + sub(DVE)."""
    shp = [z.shape[0], z.shape[1]]
    a_t = sp.tile(shp, _f32, tag="ssp_a")
    b_t = sp.tile(shp, _f32, tag="ssp_b")
    c_t = sp.tile(shp, _f32, tag="ssp_c")
    kw_a = dict(scale=KA * scale) if bias_k is None else dict(scale=KA * scale, bias=bias_k)
    kw_b = dict(scale=CA * scale) if bias_c is None else dict(scale=CA * scale, bias=bias_c)
    nc.scalar.activation(a_t[:], z, AF.Silu, **kw_a)
    nc.scalar.activation(b_t[:], z, AF.Tanh, **kw_b)
    nc.scalar.activation(c_t[:], b_t[:], AF.Square, scale=SQ)
    nc.vector.tensor_tensor(out=out_sbuf, in0=a_t[:], in1=c_t[:], op=OP.subtract)


def build(wchl):
    """Build the SPMD program. wchl[w] = chunks for window w (max over cores)."""
    nc = bacc.Bacc("TRN2", target_bir_lowering=False, debug=False, num_devices=NC)
    chbase = [0] * NW
    for w in range(1, NW):
        chbase[w] = chbase[w - 1] + wchl[w - 1]
    TCH = sum(wchl)                      # chunks per block per core
    WMAX = max(wchl)
    x0 = nc.dram_tensor("x0", [P, NA], _f32, kind="ExternalInput")
    descr = nc.dram_tensor("descr", [K, TCH * P], _f32, kind="ExternalInput")
    idxs = nc.dram_tensor("idxs", [P, TCH], _i32, kind="ExternalInput")
    offs = nc.dram_tensor("offs", [P, TCH], _f32, kind="ExternalInput")
    iota = nc.dram_tensor("iota", [P, P], _f32, kind="ExternalInput")
    wall = nc.dram_tensor("wall", [B * 13 * P, P], _f32, kind="ExternalInput")
    wg_all = nc.dram_tensor("wg_all", [B * K, P], _f32, kind="ExternalInput")
    biasT = nc.dram_tensor("biasT", [P, B * 32], _f32, kind="ExternalInput")
    xout = nc.dram_tensor("xout", [B, P, NA], _f32, kind="ExternalOutput")
    xj_full = nc.dram_tensor("xj_full", [N_ATOMS, P], _f32,
                             kind="Internal", addr_space="Shared")

    # weight row-block index within wall (per block): Wi,Wj,Wr1x3,Wr2x3,Wout,Wf1x2,Wf2x2
    def wslice(b, j):
        r = (b * 13 + j) * P
        return wall[r:r + P, :]

    # bias column index within biasT (per block, 32 slots):
    # 0:k*bi 1:c*bi 2:k*bj 3:c*bj 4..9: (k,c)*br1 r=0..2  10..12: k*(br2+fold) r
    # 13: bout 14: u 15..18: (k,c)*bf1 r=0..1  19..20: bf2 r  21: unused
    def bcol(b, j):
        return b * 32 + j

    with tile.TileContext(nc) as tc:
        with tc.tile_pool(name="pers", bufs=1) as pp, \
             tc.tile_pool(name="sp", bufs=2) as sp, \
             tc.tile_pool(name="dp", bufs=2) as dp, \
             tc.tile_pool(name="wp", bufs=2) as wp, \
             tc.tile_pool(name="gp", bufs=3) as gpool, \
             tc.tile_pool(name="ps", bufs=2, space="PSUM") as ps, \
             tc.tile_pool(name="dr", bufs=1, space="DRAM") as dr:
            x_t = pp.tile([P, NA], _f32, tag="x")
            xi_t = pp.tile([P, NA], _f32, tag="xi")
            xjt_t = pp.tile([P, NA], _f32, tag="xjt")
            m_t = pp.tile([P, NA], _f32, tag="m")
            xa_t = m_t  # xa dead before m is written; share the slot
            idx_sb = pp.tile([P, TCH], _i32, tag="idx")
            off_sb = pp.tile([P, TCH], _f32, tag="off")
            iota_sb = pp.tile([P, P], _f32, tag="iota")
            bias_sb = pp.tile([P, B * 32], _f32, tag="bias")
            ident = pp.tile([P, P], _f32, tag="ident")
            nc.sync.dma_start(out=x_t[:], in_=x0[:])
            nc.sync.dma_start(out=idx_sb[:], in_=idxs[:])
            nc.sync.dma_start(out=off_sb[:], in_=offs[:])
            nc.sync.dma_start(out=iota_sb[:], in_=iota[:])
            nc.sync.dma_start(out=bias_sb[:], in_=biasT[:])
            make_identity(nc, ident[:])

            ntile = (NA + COL_T - 1) // COL_T
            tiles = [(t * COL_T, min(COL_T, NA - t * COL_T)) for t in range(ntile)]

            def dense(dst, src, widx, bk, bc, b):
                """dst = k*ssp(src @ W + bias) tile-by-tile (transposed layout)."""
                w_sb = wp.tile([P, P], _f32, tag="w")
                nc.sync.dma_start(out=w_sb[:], in_=wslice(b, widx))
                for (c0, cn) in tiles:
                    z = ps.tile([P, COL_T], _f32, tag="z")
                    nc.tensor.matmul(z[:, :cn], lhsT=w_sb[:], rhs=src[:, c0:c0 + cn],
                                     start=True, stop=True)
                    _ssp_scaled(nc, sp, dst[:, c0:c0 + cn], z[:, :cn],
                                bias_k=bias_sb[:, bcol(b, bk):bcol(b, bk) + 1],
                                bias_c=bias_sb[:, bcol(b, bc):bcol(b, bc) + 1])

            for b in range(B):
                # xa = k*ssp(x)
                for (c0, cn) in tiles:
                    _ssp_scaled(nc, sp, xa_t[:, c0:c0 + cn], x_t[:, c0:c0 + cn])
                dense(xi_t, xa_t, 0, 0, 1, b)   # xi (scaled)
                dense(xjt_t, xa_t, 1, 2, 3, b)  # xj (scaled, transposed)
                # transpose xj to rows and write local shard of the table
                xj_loc = dr.tile([NA, P], _f32, tag="xjloc")
                for w in range(NW):
                    wa = min(P, NA - w * P)
                    tp = ps.tile([P, P], _f32, tag="tp")
                    nc.tensor.transpose(out=tp[:wa, :], in_=xjt_t[:, w * P:w * P + wa],
                                        identity=ident[:])
                    rows = sp.tile([P, P], _f32, tag="rows")
                    nc.vector.tensor_copy(out=rows[:wa, :], in_=tp[:wa, :])
                    nc.sync.dma_start(out=xj_loc[w * P:w * P + wa, :], in_=rows[:wa, :])
                nc.gpsimd.collective_compute(
                    "AllGather", OP.bypass,
                    replica_groups=[list(range(NC))],
                    ins=[xj_loc[:]], outs=[xj_full[:]])

                # pair stream
                wg_sb = wp.tile([K, P], _f32, tag="wg")
                nc.sync.dma_start(out=wg_sb[:], in_=wg_all[b * K:(b + 1) * K, :])
                for w in range(NW):
                    wa = min(P, NA - w * P)
                    w_ch = wchl[w]
                    dt = dp.tile([K, WMAX * P], _f32, tag="descr")
                    nc.sync.dma_start(
                        out=dt[:, :w_ch * P],
                        in_=descr[:, chbase[w] * P:(chbase[w] + w_ch) * P])
                    macc = ps.tile([P, P], _f32, tag="macc")
                    for c in range(w_ch):
                        ci = chbase[w] + c
                        gt = gpool.tile([P, P], _f32, tag="gt")
                        nc.gpsimd.indirect_dma_start(
                            out=gt[:], out_offset=None, in_=xj_full[:],
                            in_offset=bass.IndirectOffsetOnAxis(
                                ap=idx_sb[:, ci:ci + 1], axis=0))
                        gps = ps.tile([P, P], _f32, tag="gps")
                        nc.tensor.matmul(gps[:], lhsT=dt[:, c * P:(c + 1) * P],
                                         rhs=wg_sb[:], start=True, stop=True)
                        s_t = sp.tile([P, P], _f32, tag="s")
                        nc.vector.tensor_scalar(
                            out=s_t[:], in0=iota_sb[:],
                            scalar1=off_sb[:, ci:ci + 1], scalar2=None,
                            op0=OP.is_equal)
                        msg = sp.tile([P, P], _f32, tag="msg")
                        nc.vector.tensor_tensor(out=msg[:], in0=gps[:], in1=gt[:],
                                                op=OP.mult)
                        nc.tensor.matmul(macc[:], lhsT=msg[:], rhs=s_t[:],
                                         start=(c == 0), stop=(c == w_ch - 1))
                    nc.vector.tensor_tensor(
                        out=m_t[:, w * P:w * P + wa], in0=macc[:, :wa],
                        in1=xi_t[:, w * P:w * P + wa], op=OP.add)

                # residual (interaction) x3 — all on m (scaled)
                for r in range(NRI):
                    w1 = wp.tile([P, P], _f32, tag="w")
                    nc.sync.dma_start(out=w1[:], in_=wslice(b, 2 + r))
                    w2 = wp.tile([P, P], _f32, tag="w")
                    nc.sync.dma_start(out=w2[:], in_=wslice(b, 5 + r))
                    for (c0, cn) in tiles:
                        t1 = sp.tile([P, COL_T], _f32, tag="t1")
                        _ssp_scaled(nc, sp, t1[:, :cn], m_t[:, c0:c0 + cn],
                                    scale=1.0 / KA)  # m is k-scaled
                        z1 = ps.tile([P, COL_T], _f32, tag="z")
                        nc.tensor.matmul(z1[:, :cn], lhsT=w1[:], rhs=t1[:, :cn],
                                         start=True, stop=True)
                        t2 = sp.tile([P, COL_T], _f32, tag="t2")
                        _ssp_scaled(nc, sp, t2[:, :cn], z1[:, :cn],
                                    bias_k=bias_sb[:, bcol(b, 4 + 2 * r):bcol(b, 4 + 2 * r) + 1],
                                    bias_c=bias_sb[:, bcol(b, 5 + 2 * r):bcol(b, 5 + 2 * r) + 1])
                        z2 = ps.tile([P, COL_T], _f32, tag="z")
                        nc.tensor.matmul(z2[:, :cn], lhsT=w2[:], rhs=t2[:, :cn],
                                         start=True, stop=True)
                        t3 = sp.tile([P, COL_T], _f32, tag="t3")
                        nc.vector.tensor_scalar(
                            out=t3[:, :cn], in0=z2[:, :cn], scalar1=KA,
                            scalar2=bias_sb[:, bcol(b, 10 + r):bcol(b, 10 + r) + 1],
                            op0=OP.mult, op1=OP.add)
                        nc.vector.tensor_tensor(out=m_t[:, c0:c0 + cn],
                                                in0=m_t[:, c0:c0 + cn],
                                                in1=t3[:, :cn], op=OP.add)
                # x = u*x + ssp(m)@Wout + bout   (ssp(m) scaled; Wout pre-divided)
                wo = wp.tile([P, P], _f32, tag="w")
                nc.sync.dma_start(out=wo[:], in_=wslice(b, 8))
                for (c0, cn) in tiles:
                    mp = sp.tile([P, COL_T], _f32, tag="t1")
                    _ssp_scaled(nc, sp, mp[:, :cn], m_t[:, c0:c0 + cn], scale=1.0 / KA)
                    z = ps.tile([P, COL_T], _f32, tag="z")
                    nc.tensor.matmul(z[:, :cn], lhsT=wo[:], rhs=mp[:, :cn],
                                     start=True, stop=True)
                    ux = sp.tile([P, COL_T], _f32, tag="t2")
                    nc.vector.tensor_scalar(
                        out=ux[:, :cn], in0=x_t[:, c0:c0 + cn],
                        scalar1=bias_sb[:, bcol(b, 14):bcol(b, 14) + 1],
                        scalar2=bias_sb[:, bcol(b, 13):bcol(b, 13) + 1],
                        op0=OP.mult, op1=OP.add)
                    nc.vector.tensor_tensor(out=x_t[:, c0:c0 + cn], in0=ux[:, :cn],
                                            in1=z[:, :cn], op=OP.add)
                # residual (feature) x2 — on x (true scale)
                for r in range(NRF):
                    w1 = wp.tile([P, P], _f32, tag="w")
                    nc.sync.dma_start(out=w1[:], in_=wslice(b, 9 + r))
                    w2 = wp.tile([P, P], _f32, tag="w")
                    nc.sync.dma_start(out=w2[:], in_=wslice(b, 11 + r))  # careful map
                    for (c0, cn) in tiles:
                        t1 = sp.tile([P, COL_T], _f32, tag="t1")
                        _ssp_scaled(nc, sp, t1[:, :cn], x_t[:, c0:c0 + cn])
                        z1 = ps.tile([P, COL_T], _f32, tag="z")
                        nc.tensor.matmul(z1[:, :cn], lhsT=w1[:], rhs=t1[:, :cn],
                                         start=True, stop=True)
                        t2 = sp.tile([P, COL_T], _f32, tag="t2")
                        _ssp_scaled(nc, sp, t2[:, :cn], z1[:, :cn],
                                    bias_k=bias_sb[:, bcol(b, 15 + 2 * r):bcol(b, 15 + 2 * r) + 1],
                                    bias_c=bias_sb[:, bcol(b, 16 + 2 * r):bcol(b, 16 + 2 * r) + 1])
                        z2 = ps.tile([P, COL_T], _f32, tag="z")
                        nc.tensor.matmul(z2[:, :cn], lhsT=w2[:], rhs=t2[:, :cn],
                                         start=True, stop=True)
                        t3 = sp.tile([P, COL_T], _f32, tag="t3")
                        nc.vector.tensor_scalar(
                            out=t3[:, :cn], in0=z2[:, :cn], scalar1=1.0,
                            scalar2=bias_sb[:, bcol(b, 19 + r):bcol(b, 19 + r) + 1],
                            op0=OP.mult, op1=OP.add)
                        nc.vector.tensor_tensor(out=x_t[:, c0:c0 + cn],
                                                in0=x_t[:, c0:c0 + cn],
                                                in1=t3[:, :cn], op=OP.add)
                nc.sync.dma_start(out=xout[b, :, :], in_=x_t[:])
    nc.compile()
    return nc


def _run_spmd_timed(nc, in_maps, n_iters):
    """Mirror bass2jax.run_bass_via_pjrt's multi-core path, but keep inputs
    device-resident and time repeat executions (outputs are donated, so each
    timing iteration gets fresh device-placed zero buffers outside the timed
    region). Returns (per_core_results, best_exec_ns)."""
    import jax
    from jax.experimental.shard_map import shard_map
    from jax.sharding import Mesh, PartitionSpec, NamedSharding
    from concourse import bass2jax as b2j

    b2j.install_neuronx_cc_hook()
    partition_name = (nc.partition_id_tensor.name
                      if nc.partition_id_tensor else None)
    in_names, out_names, out_avals, zero_shapes = [], [], [], []
    for alloc in nc.m.functions[0].allocations:
        if not isinstance(alloc, mybir.MemoryLocationSet):
            continue
        name = alloc.memorylocations[0].name
        if alloc.kind == "ExternalInput":
            if name != partition_name:
                in_names.append(name)
        elif alloc.kind == "ExternalOutput":
            shape = tuple(alloc.tensor_shape)
            dtype = mybir.dt.np(alloc.dtype)
            out_names.append(name)
            out_avals.append(jax.core.ShapedArray(shape, dtype))
            zero_shapes.append((shape, dtype))
    n_params = len(in_names)
    n_outs = len(out_avals)
    in_names.extend(out_names)
    if partition_name is not None:
        in_names.append(partition_name)

    def _body(*args):
        operands = list(args)
        if partition_name is not None:
            operands.append(b2j.partition_id_tensor())
        outs = b2j._bass_exec_p.bind(
            *operands, out_avals=tuple(out_avals), in_names=tuple(in_names),
            out_names=tuple(out_names), lowering_input_output_aliases=(),
            sim_require_finite=True, sim_require_nnan=True, nc=nc)
        return tuple(outs)

    donate = tuple(range(n_params, n_params + n_outs))
    devices = jax.devices()[:NC]
    mesh = Mesh(np.asarray(devices), ("core",))
    specs = (PartitionSpec("core"),)
    sharded = jax.jit(
        shard_map(_body, mesh=mesh, in_specs=specs * (n_params + n_outs),
                  out_specs=specs * n_outs, check_rep=False),
        donate_argnums=donate, keep_unused=True)
    sh = NamedSharding(mesh, PartitionSpec("core"))
    names = in_names[:n_params]
    dev_in = [jax.device_put(
        np.concatenate([np.asarray(m[nm]) for m in in_maps], axis=0), sh)
        for nm in names]
    for a in dev_in:
        a.block_until_ready()

    def fresh_zeros():
        zs = [jax.device_put(np.zeros((NC * s[0], *s[1:]), d), sh)
              for s, d in zero_shapes]
        for z in zs:
            z.block_until_ready()
        return zs

    outs = sharded(*dev_in, *fresh_zeros())
    results_np = [np.asarray(o) for o in outs]
    times = []
    for _ in range(n_iters):
        zs = fresh_zeros()
        t0 = _time.perf_counter()
        o2 = sharded(*dev_in, *zs)
        for o in o2:
            o.block_until_ready()
        times.append(_time.perf_counter() - t0)
        del o2
    exec_ns = int(min(times) * 1e9) if times else -1
    if times:
        print("timed iters (ms):", [round(t * 1e3, 3) for t in times])
    per_core = [
        {nm: results_np[i].reshape(NC, *out_avals[i].shape)[c]
         for i, nm in enumerate(out_names)}
        for c in range(NC)
    ]
    return per_core, exec_ns


def kernel(**inputs):
    feats = np.asarray(inputs["features"], np.float32)
    cutoffs = np.asarray(inputs["cutoffs"], np.float32)
    rbfs = np.asarray(inputs["rbfs"], np.float32)
    idx_i = np.asarray(inputs["idx_i"]).astype(np.int64)
    idx_j = np.asarray(inputs["idx_j"]).astype(np.int64)
    W = {k: np.asarray(inputs[k], np.float32) for k in
         ["Wg", "Wi", "bi", "Wj", "bj", "Wr1", "br1", "Wr2", "br2",
          "Wout", "bout", "u", "Wf1", "bf1", "Wf2", "bf2"]}
    CC = -8.9582e-4  # ssp approx constant; folded into consumer biases below

    descr_full = cutoffs[:, None] * rbfs                      # [Pairs, K]

    # ---- shard pairs by destination atom core & window; compute budget ----
    bounds = np.searchsorted(idx_i, np.arange(0, N_ATOMS + 1, NA))
    win_of = (idx_i % NA) // P                               # window within core
    cnts = np.zeros((NC, NW), np.int64)
    for c in range(NC):
        s, e = bounds[c], bounds[c + 1]
        cnts[c] = np.bincount(win_of[s:e], minlength=NW)
    wchl = np.maximum(np.ceil(cnts.max(axis=0) / P).astype(np.int64), 1)
    chbase = np.zeros(NW, np.int64)
    np.cumsum(wchl[:-1], out=chbase[1:])
    TCH = int(wchl.sum())

    in_maps = []
    for c in range(NC):
        s, e = bounds[c], bounds[c + 1]
        d = np.zeros((TCH * P, K), np.float32)
        ji = np.zeros((TCH * P,), np.int32)
        of = np.zeros((TCH * P,), np.float32)
        pos = s
        for w in range(NW):
            n = cnts[c, w]
            base = chbase[w] * P
            d[base:base + n] = descr_full[pos:pos + n]
            ji[base:base + n] = idx_j[pos:pos + n]
            of[base:base + n] = (idx_i[pos:pos + n] % NA) - w * P
            pos += n
        # device layouts
        descr_t = np.ascontiguousarray(d.T)                   # [K, TCH*P]
        idx_t = np.ascontiguousarray(ji.reshape(TCH, P).T)    # [P, TCH]
        off_t = np.ascontiguousarray(of.reshape(TCH, P).T)    # [P, TCH]
        x0 = np.ascontiguousarray(feats[c * NA:(c + 1) * NA].T)
        in_maps.append(dict(x0=x0, descr=descr_t, idxs=idx_t, offs=off_t))

    iota = np.broadcast_to(np.arange(P, dtype=np.float32), (P, P)).copy()
    # ---- weights: fold 1/KA into consumers of scaled activations ----
    inv = 1.0 / KA
    wall = np.zeros((B, 13, P, P), np.float32)
    wg_all = np.zeros((B, K, P), np.float32)
    biasT = np.zeros((B, 32, P), np.float32)
    for b in range(B):
        wall[b, 0] = W["Wi"][b] * inv
        wall[b, 1] = W["Wj"][b] * inv
        for r in range(NRI):
            wall[b, 2 + r] = W["Wr1"][b, r] * inv
            wall[b, 5 + r] = W["Wr2"][b, r] * inv
        wall[b, 8] = W["Wout"][b] * inv
        for r in range(NRF):
            wall[b, 9 + r] = W["Wf1"][b, r] * inv
            wall[b, 11 + r] = W["Wf2"][b, r] * inv
        wg_all[b] = W["Wg"][b]
        bi_e = W["bi"][b] + CC * W["Wi"][b].sum(0)
        bj_e = W["bj"][b] + CC * W["Wj"][b].sum(0)
        biasT[b, 0] = KA * bi_e
        biasT[b, 1] = CA * bi_e
        biasT[b, 2] = KA * bj_e
        biasT[b, 3] = CA * bj_e
        for r in range(NRI):
            br1_e = W["br1"][b, r] + CC * W["Wr1"][b, r].sum(0)
            br2_e = W["br2"][b, r] + CC * W["Wr2"][b, r].sum(0)
            biasT[b, 4 + 2 * r] = KA * br1_e
            biasT[b, 5 + 2 * r] = CA * br1_e
            biasT[b, 10 + r] = KA * br2_e
        biasT[b, 13] = W["bout"][b] + CC * W["Wout"][b].sum(0)
        biasT[b, 14] = W["u"][b]
        for r in range(NRF):
            bf1_e = W["bf1"][b, r] + CC * W["Wf1"][b, r].sum(0)
            biasT[b, 15 + 2 * r] = KA * bf1_e
            biasT[b, 16 + 2 * r] = CA * bf1_e
            biasT[b, 19 + r] = W["bf2"][b, r] + CC * W["Wf2"][b, r].sum(0)
    shared = dict(iota=iota,
                  wall=wall.reshape(B * 13 * P, P),
                  wg_all=wg_all.reshape(B * K, P),
                  biasT=np.ascontiguousarray(
                      biasT.reshape(B * 32, P).T))
    for m in in_maps:
        m.update(shared)

    nc = build([int(v) for v in wchl])
    global LAST_EXEC_NS
    n_iters = int(os.environ.get("PHYS_TIME_ITERS", "0"))
    if n_iters > 0:
        results, LAST_EXEC_NS = _run_spmd_timed(nc, in_maps, n_iters)
    else:
        res = bass_utils.run_bass_kernel_spmd(
            nc, in_maps, core_ids=list(range(NC)))
        results = res.results
        LAST_EXEC_NS = res.exec_time_ns if res.exec_time_ns is not None else -1
    out = np.empty((B, N_ATOMS, F), np.float32)
    for c in range(NC):
        slab = results[c]["xout"]              # [B, 128, NA]
        out[:, c * NA:(c + 1) * NA, :] = np.transpose(slab, (0, 2, 1))
    return out

